# revision 2
# baseline (speedup 1.0000x reference)
"""Causal self-attention (B=4, T=2048, C=2048, H=16, D=128) on 8 TRN2 NeuronCores.

Sharding: 8 shards = (batch b in 0..3) x (head-group hg in {0,1}, 8 heads each).
Each core computes qkv for its (batch, 8 heads), causal attention, and a
partial output projection y_hg @ w_proj[hg-rows]; the host sums the two
partials per batch and adds b_proj.

Device compute is bf16 (fp32 PSUM accumulation); inputs are cast/sliced/
pre-transposed on the host so the device never transposes anything:
  - x is shipped transposed (xT, [C, T] tiled as [128, 16, 2048]).
  - qT/kT computed as w.T @ xT (output [head-dim, T]) -> directly usable as
    matmul operands for S^T = k.T-tile @ qT.
  - softmax runs on S^T (keys on partitions): exp on ScalarE (no max
    subtraction needed; logits are O(1)), causal masking via precomputed 0/1
    mask tiles, denominators via ones-matmul column sums, normalization folded
    into the yT eviction with a K=1 broadcast matmul of the reciprocals.
  - v computed in natural [T, head-dim] layout (lhsT = xT tiles) so the AV
    matmul yT += v-block.T @ P^T needs no transpose either.
  - out = y @ w_proj with lhsT = yT tiles, rhs = w_proj slices.
1/sqrt(D) is folded into w_q (and b_q) on the host.
"""

import math
from contextlib import ExitStack

import ml_dtypes
import numpy as np

import concourse.bass as bass  # noqa: F401  (bass types used via tile/bacc)
import concourse.tile as tile
from concourse import bacc, mybir
from concourse.alu_op_type import AluOpType
from concourse.bass_utils import run_bass_kernel_spmd

P = 128
B, T, C, H, D = 4, 2048, 2048, 16, 128
HG = 2              # head groups (tensor-parallel factor); B * HG = 8 cores
HL = H // HG        # heads per core
KT = C // P         # 16 contraction tiles
TCH = T // 512      # 4 query chunks of 512
BF16 = mybir.dt.bfloat16
F32 = mybir.dt.float32
EXP = mybir.ActivationFunctionType.Exp

_NC_CACHE = []
LAST_RESULTS = None  # BassKernelResults of the most recent kernel() call


def build_nc():
    nc = bacc.Bacc("TRN2", target_bir_lowering=False, debug=False, num_devices=8)

    xt_d = nc.dram_tensor("xt", [P, KT, T], BF16, kind="ExternalInput")
    wq_d = nc.dram_tensor("wq", [HL, P, KT, P], BF16, kind="ExternalInput")
    wk_d = nc.dram_tensor("wk", [HL, P, KT, P], BF16, kind="ExternalInput")
    wv_d = nc.dram_tensor("wv", [P, KT, HL * D], BF16, kind="ExternalInput")
    wp_d = nc.dram_tensor("wp", [P, HL, C], BF16, kind="ExternalInput")
    bq_d = nc.dram_tensor("bq", [P, HL], F32, kind="ExternalInput")
    bk_d = nc.dram_tensor("bk", [P, HL], F32, kind="ExternalInput")
    bv_d = nc.dram_tensor("bv", [P, HL * D], F32, kind="ExternalInput")
    mk_d = nc.dram_tensor("masks", [P, 4, 512], BF16, kind="ExternalInput")
    out_d = nc.dram_tensor("out", [T, C], F32, kind="ExternalOutput")

    with nc.allow_low_precision("bf16 attention kernel"), \
         tile.TileContext(nc) as tc, ExitStack() as ctx:
        consts = ctx.enter_context(tc.tile_pool(name="consts", bufs=1))
        big = ctx.enter_context(tc.tile_pool(name="big", bufs=1))      # xt -> wp
        vpool = ctx.enter_context(tc.tile_pool(name="v", bufs=1))
        mid = ctx.enter_context(tc.tile_pool(name="mid", bufs=1))      # wv -> yt
        qk = ctx.enter_context(tc.tile_pool(name="qk", bufs=2))
        wcol = ctx.enter_context(tc.tile_pool(name="wcol", bufs=4))
        ppool = ctx.enter_context(tc.tile_pool(name="p", bufs=6))
        spool = ctx.enter_context(tc.tile_pool(name="small", bufs=4))
        opool = ctx.enter_context(tc.tile_pool(name="o", bufs=4))
        mm = ctx.enter_context(tc.tile_pool(name="mm", bufs=4, space="PSUM"))
        psy = ctx.enter_context(tc.tile_pool(name="psy", bufs=2, space="PSUM"))
        psd = ctx.enter_context(tc.tile_pool(name="psd", bufs=2, space="PSUM"))

        # constants
        bq_sb = consts.tile([P, HL], F32)
        nc.sync.dma_start(bq_sb[:], bq_d[:])
        bk_sb = consts.tile([P, HL], F32)
        nc.sync.dma_start(bk_sb[:], bk_d[:])
        bv_sb = consts.tile([P, HL * D], F32)
        nc.sync.dma_start(bv_sb[:], bv_d[:])
        mk_sb = consts.tile([P, 4, 512], BF16)
        nc.sync.dma_start(mk_sb[:], mk_d[:])
        ones_col = consts.tile([P, 1], BF16)
        nc.vector.memset(ones_col[:], 1.0)
        ones_row = consts.tile([1, P], BF16)
        nc.vector.memset(ones_row[:], 1.0)

        # load xT (strip per kt so compute can start early) and wv
        xt = big.tile([P, KT, T], BF16, tag="big")
        for kt in range(KT):
            nc.sync.dma_start(xt[:, kt, :], xt_d[:, kt, :])
        wv = mid.tile([P, KT, HL * D], BF16, tag="mid")
        nc.sync.dma_start(wv[:], wv_d[:])

        # stage B: v = x @ w_v, natural layout v_sb[p=T within tt, tt, hl*D]
        v_sb = vpool.tile([P, KT, HL * D], BF16)
        for tt in range(KT):
            pss = [mm.tile([P, 512], F32, tag="mm", name=f"mmps{_}") for _ in range(2)]
            for kt in range(KT):
                for wc in range(2):
                    nc.tensor.matmul(
                        pss[wc][:],
                        xt[:, kt, tt * P:(tt + 1) * P],
                        wv[:, kt, wc * 512:(wc + 1) * 512],
                        start=(kt == 0), stop=(kt == KT - 1),
                    )
            for wc in range(2):
                nc.vector.tensor_tensor(
                    v_sb[:, tt, wc * 512:(wc + 1) * 512], pss[wc][:],
                    bv_sb[:, wc * 512:(wc + 1) * 512], AluOpType.add,
                )

        yt = mid.tile([P, HL, T], BF16, tag="mid")  # waits until wv is dead

        for h in range(HL):
            # qT/kT for head h: [D=128, T], q pre-scaled by 1/sqrt(D) via host
            qh = qk.tile([P, T], BF16, tag="qh")
            kh = qk.tile([P, T], BF16, tag="kh")
            for dst, w_d, b_sb in ((qh, wq_d, bq_sb), (kh, wk_d, bk_sb)):
                wc_t = wcol.tile([P, KT, P], BF16)
                nc.sync.dma_start(wc_t[:], w_d[h])
                for pair in range(2):
                    pss = [mm.tile([P, 512], F32, tag="mm", name=f"mmps{_}") for _ in range(2)]
                    for kt in range(KT):
                        for t2 in range(2):
                            tci = pair * 2 + t2
                            nc.tensor.matmul(
                                pss[t2][:],
                                wc_t[:, kt, :],
                                xt[:, kt, tci * 512:(tci + 1) * 512],
                                start=(kt == 0), stop=(kt == KT - 1),
                            )
                    for t2 in range(2):
                        tci = pair * 2 + t2
                        nc.vector.tensor_tensor(
                            dst[:, tci * 512:(tci + 1) * 512], pss[t2][:],
                            b_sb[:, h:h + 1].to_broadcast((P, 512)),
                            AluOpType.add,
                        )

            # attention for head h, query chunks of 512
            for c in range(TCH):
                jmax = 4 * c + 3
                ps_y = psy.tile([P, 512], F32)
                ps_d = psd.tile([1, 512], F32)
                for jt in range(jmax + 1):
                    ps_s = mm.tile([P, 512], F32, tag="mm")
                    nc.tensor.matmul(
                        ps_s[:], kh[:, jt * P:(jt + 1) * P],
                        qh[:, c * 512:(c + 1) * 512], start=True, stop=True,
                    )
                    pt = ppool.tile([P, 512], BF16)
                    nc.scalar.activation(pt[:], ps_s[:], EXP)
                    if jt >= 4 * c:  # diagonal block: causal 0/1 mask
                        nc.vector.tensor_tensor(
                            pt[:], pt[:], mk_sb[:, jt - 4 * c, :], AluOpType.mult
                        )
                    nc.tensor.matmul(
                        ps_y[:], v_sb[:, jt, h * D:(h + 1) * D], pt[:],
                        start=(jt == 0), stop=(jt == jmax),
                    )
                    nc.tensor.matmul(
                        ps_d[:], ones_col[:], pt[:],
                        start=(jt == 0), stop=(jt == jmax),
                    )
                r32 = spool.tile([1, 512], F32, tag="r32")
                nc.vector.reciprocal(r32[:], ps_d[:])
                rb = spool.tile([1, 512], BF16, tag="rb")
                nc.vector.tensor_copy(rb[:], r32[:])
                ps_b = mm.tile([P, 512], F32, tag="mm")
                nc.tensor.matmul(ps_b[:], ones_row[:], rb[:], start=True, stop=True)
                bc = spool.tile([P, 512], F32, tag="bc")
                nc.scalar.copy(bc[:], ps_b[:])
                nc.vector.tensor_tensor(
                    yt[:, h, c * 512:(c + 1) * 512], ps_y[:], bc[:], AluOpType.mult
                )

        # stage D: partial out = y @ w_proj[hg rows]
        wp = big.tile([P, HL, C], BF16, tag="big")  # waits until xt is dead
        nc.sync.dma_start(wp[:], wp_d[:])
        for tt in range(KT):
            for pair in range(2):
                pss = [mm.tile([P, 512], F32, tag="mm", name=f"mmps{_}") for _ in range(2)]
                for hh in range(HL):
                    for c2 in range(2):
                        cc = pair * 2 + c2
                        nc.tensor.matmul(
                            pss[c2][:],
                            yt[:, hh, tt * P:(tt + 1) * P],
                            wp[:, hh, cc * 512:(cc + 1) * 512],
                            start=(hh == 0), stop=(hh == HL - 1),
                        )
                for c2 in range(2):
                    cc = pair * 2 + c2
                    ot = opool.tile([P, 512], F32)
                    nc.scalar.copy(ot[:], pss[c2][:])
                    nc.sync.dma_start(
                        out_d[tt * P:(tt + 1) * P, cc * 512:(cc + 1) * 512], ot[:]
                    )

    nc.compile()
    return nc


def get_nc():
    if not _NC_CACHE:
        _NC_CACHE.append(build_nc())
    return _NC_CACHE[0]


def _bf(a):
    return np.ascontiguousarray(a).astype(ml_dtypes.bfloat16)


def _shard_inputs(x, w_attn, b_attn, w_proj):
    """Build the 8 per-core in_maps."""
    scale = 1.0 / math.sqrt(D)
    w_q, w_k, w_v = w_attn[:, :C], w_attn[:, C:2 * C], w_attn[:, 2 * C:]
    b_q, b_k, b_v = b_attn[:C], b_attn[C:2 * C], b_attn[2 * C:]

    # causal masks for the 4 j-tile positions within a 512 diagonal block
    j = np.arange(P)[:, None]
    i = np.arange(512)[None, :]
    masks = np.stack([(j + P * p <= i) for p in range(4)]).astype(np.float32)
    masks_bf = _bf(masks.transpose(1, 0, 2))  # [128, 4, 512]

    per_hg = {}
    for hg in range(HG):
        s = slice(hg * HL * D, (hg + 1) * HL * D)
        wq = _bf((w_q[:, s] * scale).reshape(KT, P, HL, D).transpose(2, 1, 0, 3))
        wk = _bf(w_k[:, s].reshape(KT, P, HL, D).transpose(2, 1, 0, 3))
        wv = _bf(w_v[:, s].reshape(KT, P, HL * D).transpose(1, 0, 2))
        wp = _bf(w_proj[s, :].reshape(HL, P, C).transpose(1, 0, 2))
        bq = np.ascontiguousarray(
            (b_q[s] * scale).reshape(HL, P).T).astype(np.float32)
        bk = np.ascontiguousarray(b_k[s].reshape(HL, P).T).astype(np.float32)
        bv = np.ascontiguousarray(
            np.broadcast_to(b_v[s], (P, HL * D))).astype(np.float32)
        per_hg[hg] = dict(wq=wq, wk=wk, wv=wv, wp=wp, bq=bq, bk=bk, bv=bv)

    in_maps = []
    for core in range(8):
        b, hg = core // HG, core % HG
        xt = _bf(x[b].T.reshape(KT, P, T).transpose(1, 0, 2))
        in_maps.append({"xt": xt, "masks": masks_bf, **per_hg[hg]})
    return in_maps


def kernel(x, w_attn, b_attn, w_proj, b_proj):
    global LAST_RESULTS
    x = np.asarray(x, dtype=np.float32)
    w_attn = np.asarray(w_attn, dtype=np.float32)
    b_attn = np.asarray(b_attn, dtype=np.float32)
    w_proj = np.asarray(w_proj, dtype=np.float32)
    b_proj = np.asarray(b_proj, dtype=np.float32)

    nc = get_nc()
    in_maps = _shard_inputs(x, w_attn, b_attn, w_proj)
    res = run_bass_kernel_spmd(nc, in_maps, core_ids=list(range(8)))
    LAST_RESULTS = res

    out = np.empty((B, T, C), dtype=np.float32)
    for b in range(B):
        out[b] = res.results[HG * b]["out"] + res.results[HG * b + 1]["out"]
        out[b] += b_proj[None, :]
    return out


# revision 4
# speedup vs baseline: 1.0772x; 1.0772x over previous
"""Causal self-attention (B=4, T=2048, C=2048, H=16, D=128) on 8 TRN2 NeuronCores.

Sharding: 8 shards = (batch b in 0..3) x (head-group hg in {0,1}, 8 heads each).
Each core computes qkv for its (batch, 8 heads), causal attention, and a
partial output projection y_hg @ w_proj[hg-rows]; the host sums the two
partials per batch and adds b_proj.

Device compute is bf16 (fp32 PSUM accumulation); inputs are cast/sliced/
pre-transposed on the host so the device never transposes anything:
  - x is shipped transposed (xT, [C, T] tiled as [128, 16, 2048]).
  - qT/kT computed as w.T @ xT (output [head-dim, T]) -> directly usable as
    matmul operands for S^T = k.T-tile @ qT.
  - softmax runs on S^T (keys on partitions): exp on ScalarE (no max
    subtraction needed; logits are O(1)), causal masking via precomputed 0/1
    mask tiles, denominators via ones-matmul column sums, normalization folded
    into the yT eviction with a K=1 broadcast matmul of the reciprocals.
  - v computed in natural [T, head-dim] layout (lhsT = xT tiles) so the AV
    matmul yT += v-block.T @ P^T needs no transpose either.
  - out = y @ w_proj with lhsT = yT tiles, rhs = w_proj slices.
1/sqrt(D) is folded into w_q (and b_q) on the host.
"""

import math
from contextlib import ExitStack

import ml_dtypes
import numpy as np

import concourse.bass as bass  # noqa: F401  (bass types used via tile/bacc)
import concourse.tile as tile
from concourse import bacc, mybir
from concourse.alu_op_type import AluOpType
from concourse.bass_utils import run_bass_kernel_spmd

P = 128
B, T, C, H, D = 4, 2048, 2048, 16, 128
HG = 2              # head groups (tensor-parallel factor); B * HG = 8 cores
HL = H // HG        # heads per core
KT = C // P         # 16 contraction tiles
TCH = T // 512      # 4 query chunks of 512
BF16 = mybir.dt.bfloat16
F32 = mybir.dt.float32
EXP = mybir.ActivationFunctionType.Exp

_NC_CACHE = []
LAST_RESULTS = None  # BassKernelResults of the most recent kernel() call


def build_nc():
    nc = bacc.Bacc("TRN2", target_bir_lowering=False, debug=False, num_devices=8)

    xt_d = nc.dram_tensor("xt", [P, KT, T], BF16, kind="ExternalInput")
    wq_d = nc.dram_tensor("wq", [HL, P, KT, P], BF16, kind="ExternalInput")
    wk_d = nc.dram_tensor("wk", [HL, P, KT, P], BF16, kind="ExternalInput")
    wv_d = nc.dram_tensor("wv", [P, KT, HL * D], BF16, kind="ExternalInput")
    wp_d = nc.dram_tensor("wp", [P, HL, C], BF16, kind="ExternalInput")
    bq_d = nc.dram_tensor("bq", [P, HL], F32, kind="ExternalInput")
    bk_d = nc.dram_tensor("bk", [P, HL], F32, kind="ExternalInput")
    bv_d = nc.dram_tensor("bv", [P, HL * D], F32, kind="ExternalInput")
    mk_d = nc.dram_tensor("masks", [P, 4, 512], BF16, kind="ExternalInput")
    out_d = nc.dram_tensor("out", [T, C], F32, kind="ExternalOutput")

    with nc.allow_low_precision("bf16 attention kernel"), \
         tile.TileContext(nc) as tc, ExitStack() as ctx:
        consts = ctx.enter_context(tc.tile_pool(name="consts", bufs=1))
        big = ctx.enter_context(tc.tile_pool(name="big", bufs=1))      # xt -> wp
        vpool = ctx.enter_context(tc.tile_pool(name="v", bufs=1))
        mid = ctx.enter_context(tc.tile_pool(name="mid", bufs=1))      # wv -> yt
        qk = ctx.enter_context(tc.tile_pool(name="qk", bufs=2))
        wcol = ctx.enter_context(tc.tile_pool(name="wcol", bufs=4))
        ppool = ctx.enter_context(tc.tile_pool(name="p", bufs=6))
        spool = ctx.enter_context(tc.tile_pool(name="small", bufs=4))
        opool = ctx.enter_context(tc.tile_pool(name="o", bufs=4))
        mm = ctx.enter_context(tc.tile_pool(name="mm", bufs=4, space="PSUM"))
        psy = ctx.enter_context(tc.tile_pool(name="psy", bufs=2, space="PSUM"))
        psd = ctx.enter_context(tc.tile_pool(name="psd", bufs=2, space="PSUM"))

        # constants
        bq_sb = consts.tile([P, HL], F32)
        nc.sync.dma_start(bq_sb[:], bq_d[:])
        bk_sb = consts.tile([P, HL], F32)
        nc.sync.dma_start(bk_sb[:], bk_d[:])
        bv_sb = consts.tile([P, HL * D], F32)
        nc.sync.dma_start(bv_sb[:], bv_d[:])
        mk_sb = consts.tile([P, 4, 512], BF16)
        nc.sync.dma_start(mk_sb[:], mk_d[:])
        ones_col = consts.tile([P, 1], BF16)
        nc.vector.memset(ones_col[:], 1.0)
        ones_row = consts.tile([1, P], BF16)
        nc.vector.memset(ones_row[:], 1.0)

        # load wv first, then xT strips, so stage B can start as strips arrive
        wv = mid.tile([P, KT, HL * D], BF16, tag="mid")
        nc.sync.dma_start(wv[:], wv_d[:])
        xt = big.tile([P, KT, T], BF16, tag="big")
        for kt in range(KT):
            nc.sync.dma_start(xt[:, kt, :], xt_d[:, kt, :])

        # stage B: v = x @ w_v, natural layout v_sb[p=T within tt, tt, hl*D]
        v_sb = vpool.tile([P, KT, HL * D], BF16)
        for tt in range(KT):
            pss = [mm.tile([P, 512], F32, tag="mm", name=f"mmps{_}") for _ in range(2)]
            for kt in range(KT):
                for wc in range(2):
                    nc.tensor.matmul(
                        pss[wc][:],
                        xt[:, kt, tt * P:(tt + 1) * P],
                        wv[:, kt, wc * 512:(wc + 1) * 512],
                        start=(kt == 0), stop=(kt == KT - 1),
                    )
            for wc in range(2):
                nc.vector.tensor_tensor(
                    v_sb[:, tt, wc * 512:(wc + 1) * 512], pss[wc][:],
                    bv_sb[:, wc * 512:(wc + 1) * 512], AluOpType.add,
                )

        yt = mid.tile([P, HL, T], BF16, tag="mid")  # waits until wv is dead

        for h in range(HL):
            # qT/kT for head h: [D=128, T], q pre-scaled by 1/sqrt(D) via host
            qh = qk.tile([P, T], BF16, tag="qh")
            kh = qk.tile([P, T], BF16, tag="kh")
            for dst, w_d, b_sb in ((qh, wq_d, bq_sb), (kh, wk_d, bk_sb)):
                wc_t = wcol.tile([P, KT, P], BF16)
                nc.sync.dma_start(wc_t[:], w_d[h])
                for pair in range(2):
                    pss = [mm.tile([P, 512], F32, tag="mm", name=f"mmps{_}") for _ in range(2)]
                    for kt in range(KT):
                        for t2 in range(2):
                            tci = pair * 2 + t2
                            nc.tensor.matmul(
                                pss[t2][:],
                                wc_t[:, kt, :],
                                xt[:, kt, tci * 512:(tci + 1) * 512],
                                start=(kt == 0), stop=(kt == KT - 1),
                            )
                    for t2 in range(2):
                        tci = pair * 2 + t2
                        nc.vector.tensor_tensor(
                            dst[:, tci * 512:(tci + 1) * 512], pss[t2][:],
                            b_sb[:, h:h + 1].to_broadcast((P, 512)),
                            AluOpType.add,
                        )

            # attention for head h, query chunks of 512
            for c in range(TCH):
                jmax = 4 * c + 3
                ps_y = psy.tile([P, 512], F32)
                ps_d = psd.tile([1, 512], F32)
                for jt in range(jmax + 1):
                    ps_s = mm.tile([P, 512], F32, tag="mm")
                    nc.tensor.matmul(
                        ps_s[:], kh[:, jt * P:(jt + 1) * P],
                        qh[:, c * 512:(c + 1) * 512], start=True, stop=True,
                    )
                    pt = ppool.tile([P, 512], BF16)
                    nc.scalar.activation(pt[:], ps_s[:], EXP)
                    if jt >= 4 * c:  # diagonal block: causal 0/1 mask
                        nc.vector.tensor_tensor(
                            pt[:], pt[:], mk_sb[:, jt - 4 * c, :], AluOpType.mult
                        )
                    nc.tensor.matmul(
                        ps_y[:], v_sb[:, jt, h * D:(h + 1) * D], pt[:],
                        start=(jt == 0), stop=(jt == jmax),
                    )
                    nc.tensor.matmul(
                        ps_d[:], ones_col[:], pt[:],
                        start=(jt == 0), stop=(jt == jmax),
                    )
                # Decouple normalization from the PE pipeline: evict both PSUM
                # accumulators to SBUF right away (frees psy/psd slots), then
                # broadcast the raw denominators with a K=1 matmul and do the
                # reciprocal full-width off the critical path.
                yu = spool.tile([P, 512], F32, tag="yu")
                nc.scalar.copy(yu[:], ps_y[:])
                dn = spool.tile([1, 512], BF16, tag="dn")
                nc.scalar.copy(dn[:], ps_d[:])
                ps_b = mm.tile([P, 512], F32, tag="mm")
                nc.tensor.matmul(ps_b[:], ones_row[:], dn[:], start=True, stop=True)
                rc = spool.tile([P, 512], F32, tag="rc")
                nc.vector.reciprocal(rc[:], ps_b[:])
                nc.vector.tensor_tensor(
                    yt[:, h, c * 512:(c + 1) * 512], yu[:], rc[:], AluOpType.mult
                )

        # stage D: partial out = y @ w_proj[hg rows]
        wp = big.tile([P, HL, C], BF16, tag="big")  # waits until xt is dead
        nc.sync.dma_start(wp[:], wp_d[:])
        for tt in range(KT):
            for pair in range(2):
                pss = [mm.tile([P, 512], F32, tag="mm", name=f"mmps{_}") for _ in range(2)]
                for hh in range(HL):
                    for c2 in range(2):
                        cc = pair * 2 + c2
                        nc.tensor.matmul(
                            pss[c2][:],
                            yt[:, hh, tt * P:(tt + 1) * P],
                            wp[:, hh, cc * 512:(cc + 1) * 512],
                            start=(hh == 0), stop=(hh == HL - 1),
                        )
                for c2 in range(2):
                    cc = pair * 2 + c2
                    ot = opool.tile([P, 512], F32)
                    nc.scalar.copy(ot[:], pss[c2][:])
                    nc.sync.dma_start(
                        out_d[tt * P:(tt + 1) * P, cc * 512:(cc + 1) * 512], ot[:]
                    )

    nc.compile()
    return nc


def get_nc():
    if not _NC_CACHE:
        _NC_CACHE.append(build_nc())
    return _NC_CACHE[0]


def _bf(a):
    return np.ascontiguousarray(a).astype(ml_dtypes.bfloat16)


def _shard_inputs(x, w_attn, b_attn, w_proj):
    """Build the 8 per-core in_maps."""
    scale = 1.0 / math.sqrt(D)
    w_q, w_k, w_v = w_attn[:, :C], w_attn[:, C:2 * C], w_attn[:, 2 * C:]
    b_q, b_k, b_v = b_attn[:C], b_attn[C:2 * C], b_attn[2 * C:]

    # causal masks for the 4 j-tile positions within a 512 diagonal block
    j = np.arange(P)[:, None]
    i = np.arange(512)[None, :]
    masks = np.stack([(j + P * p <= i) for p in range(4)]).astype(np.float32)
    masks_bf = _bf(masks.transpose(1, 0, 2))  # [128, 4, 512]

    per_hg = {}
    for hg in range(HG):
        s = slice(hg * HL * D, (hg + 1) * HL * D)
        wq = _bf((w_q[:, s] * scale).reshape(KT, P, HL, D).transpose(2, 1, 0, 3))
        wk = _bf(w_k[:, s].reshape(KT, P, HL, D).transpose(2, 1, 0, 3))
        wv = _bf(w_v[:, s].reshape(KT, P, HL * D).transpose(1, 0, 2))
        wp = _bf(w_proj[s, :].reshape(HL, P, C).transpose(1, 0, 2))
        bq = np.ascontiguousarray(
            (b_q[s] * scale).reshape(HL, P).T).astype(np.float32)
        bk = np.ascontiguousarray(b_k[s].reshape(HL, P).T).astype(np.float32)
        bv = np.ascontiguousarray(
            np.broadcast_to(b_v[s], (P, HL * D))).astype(np.float32)
        per_hg[hg] = dict(wq=wq, wk=wk, wv=wv, wp=wp, bq=bq, bk=bk, bv=bv)

    in_maps = []
    for core in range(8):
        b, hg = core // HG, core % HG
        xt = _bf(x[b].T.reshape(KT, P, T).transpose(1, 0, 2))
        in_maps.append({"xt": xt, "masks": masks_bf, **per_hg[hg]})
    return in_maps


def kernel(x, w_attn, b_attn, w_proj, b_proj):
    global LAST_RESULTS
    x = np.asarray(x, dtype=np.float32)
    w_attn = np.asarray(w_attn, dtype=np.float32)
    b_attn = np.asarray(b_attn, dtype=np.float32)
    w_proj = np.asarray(w_proj, dtype=np.float32)
    b_proj = np.asarray(b_proj, dtype=np.float32)

    nc = get_nc()
    in_maps = _shard_inputs(x, w_attn, b_attn, w_proj)
    res = run_bass_kernel_spmd(nc, in_maps, core_ids=list(range(8)))
    LAST_RESULTS = res

    out = np.empty((B, T, C), dtype=np.float32)
    for b in range(B):
        out[b] = res.results[HG * b]["out"] + res.results[HG * b + 1]["out"]
        out[b] += b_proj[None, :]
    return out


# revision 13
# speedup vs baseline: 1.1794x; 1.0949x over previous
"""Causal self-attention (B=4, T=2048, C=2048, H=16, D=128) on 8 TRN2 NeuronCores.

Sharding: 8 shards = (batch b in 0..3) x (head-group hg in {0,1}, 8 heads each).
Each core computes qkv for its (batch, 8 heads), causal attention, and a
partial output projection y_hg @ w_proj[hg-rows]; the host sums the two
partials per batch and adds b_proj.

Device compute is bf16 (fp32 PSUM accumulation); inputs are cast/sliced/
pre-transposed on the host so the device never transposes anything:
  - x is shipped transposed (xT, [C, T] tiled as [128, 16, 2048]).
  - qT/kT computed as w.T @ xT (output [head-dim, T]) -> directly usable as
    matmul operands for S^T = k.T-tile @ qT.
  - softmax runs on S^T (keys on partitions): exp on ScalarE (no max
    subtraction needed; logits are O(1)), causal masking via precomputed 0/1
    mask tiles, denominators via ones-matmul column sums, normalization folded
    into the yT eviction with a K=1 broadcast matmul of the reciprocals.
  - v computed in natural [T, head-dim] layout (lhsT = xT tiles) so the AV
    matmul yT += v-block.T @ P^T needs no transpose either.
  - out = y @ w_proj with lhsT = yT tiles, rhs = w_proj slices.
1/sqrt(D) is folded into w_q (and b_q) on the host.
"""

import math
from contextlib import ExitStack

import ml_dtypes
import numpy as np

import concourse.bass as bass  # noqa: F401  (bass types used via tile/bacc)
import concourse.tile as tile
from concourse import bacc, mybir
from concourse.alu_op_type import AluOpType
from concourse.bass_utils import run_bass_kernel_spmd

P = 128
B, T, C, H, D = 4, 2048, 2048, 16, 128
HG = 2              # head groups (tensor-parallel factor); B * HG = 8 cores
HL = H // HG        # heads per core
KT = C // P         # 16 contraction tiles
TCH = T // 512      # 4 query chunks of 512
BF16 = mybir.dt.bfloat16
F32 = mybir.dt.float32
EXP = mybir.ActivationFunctionType.Exp

_NC_CACHE = []
LAST_RESULTS = None  # BassKernelResults of the most recent kernel() call


def build_nc():
    nc = bacc.Bacc("TRN2", target_bir_lowering=False, debug=False, num_devices=8)

    xt_d = nc.dram_tensor("xt", [P, KT, T], BF16, kind="ExternalInput")
    wq_d = nc.dram_tensor("wq", [HL, P, KT, P], BF16, kind="ExternalInput")
    wk_d = nc.dram_tensor("wk", [HL, P, KT, P], BF16, kind="ExternalInput")
    wv_d = nc.dram_tensor("wv", [P, KT, HL * D], BF16, kind="ExternalInput")
    wp_d = nc.dram_tensor("wp", [P, HL, C], BF16, kind="ExternalInput")
    bq_d = nc.dram_tensor("bq", [P, HL], F32, kind="ExternalInput")
    bk_d = nc.dram_tensor("bk", [P, HL], F32, kind="ExternalInput")
    bv_d = nc.dram_tensor("bv", [P, HL * D], F32, kind="ExternalInput")
    mk_d = nc.dram_tensor("masks", [P, 2, 1024], BF16, kind="ExternalInput")
    out_d = nc.dram_tensor("out", [T, C], F32, kind="ExternalOutput")

    with nc.allow_low_precision("bf16 attention kernel"), \
         tile.TileContext(nc) as tc, ExitStack() as ctx:
        consts = ctx.enter_context(tc.tile_pool(name="consts", bufs=1))
        big = ctx.enter_context(tc.tile_pool(name="big", bufs=1))      # xt -> wp
        vpool = ctx.enter_context(tc.tile_pool(name="v", bufs=1))
        mid = ctx.enter_context(tc.tile_pool(name="mid", bufs=1))      # wv -> yt
        qk = ctx.enter_context(tc.tile_pool(name="qk", bufs=2))
        wcol = ctx.enter_context(tc.tile_pool(name="wcol", bufs=3))
        ppool = ctx.enter_context(tc.tile_pool(name="p", bufs=4))
        spool = ctx.enter_context(tc.tile_pool(name="small", bufs=3))
        opool = ctx.enter_context(tc.tile_pool(name="o", bufs=2))
        # PSUM: two 2-bank strips (matmul pairs) + ps_y/ps_b pool + ps_d pool
        mm = ctx.enter_context(tc.tile_pool(name="mm", bufs=2, space="PSUM"))
        psy = ctx.enter_context(tc.tile_pool(name="psy", bufs=2, space="PSUM"))
        psd = ctx.enter_context(tc.tile_pool(name="psd", bufs=2, space="PSUM"))

        # constants
        bq_sb = consts.tile([P, HL], F32)
        nc.sync.dma_start(bq_sb[:], bq_d[:])
        bk_sb = consts.tile([P, HL], F32)
        nc.sync.dma_start(bk_sb[:], bk_d[:])
        bv_sb = consts.tile([P, HL * D], F32)
        nc.sync.dma_start(bv_sb[:], bv_d[:])
        mk_sb = consts.tile([P, 2, 1024], BF16)
        nc.sync.dma_start(mk_sb[:], mk_d[:])
        ones_col = consts.tile([P, 1], BF16)
        nc.vector.memset(ones_col[:], 1.0)
        ones_row = consts.tile([1, P], BF16)
        nc.vector.memset(ones_row[:], 1.0)

        # load wv first, then xT strips, so stage B can start as strips arrive
        wv = mid.tile([P, KT, HL * D], BF16, tag="mid")
        nc.sync.dma_start(wv[:], wv_d[:])
        xt = big.tile([P, KT, T], BF16, tag="big")
        for kt in range(KT):
            nc.sync.dma_start(xt[:, kt, :], xt_d[:, kt, :])

        # stage B: v = x @ w_v, natural layout v_sb[p=T within tt, tt, hl*D]
        v_sb = vpool.tile([P, KT, HL * D], BF16)
        for tt in range(KT):
            ps = mm.tile([P, 1024], F32, tag="mm")
            for kt in range(KT):
                for wc in range(2):
                    nc.tensor.matmul(
                        ps[:, wc * 512:(wc + 1) * 512],
                        xt[:, kt, tt * P:(tt + 1) * P],
                        wv[:, kt, wc * 512:(wc + 1) * 512],
                        start=(kt == 0), stop=(kt == KT - 1),
                    )
            nc.vector.tensor_tensor(
                v_sb[:, tt, :], ps[:], bv_sb[:], AluOpType.add,
            )

        yt = mid.tile([P, HL, T], BF16, tag="mid")  # waits until wv is dead

        for h in range(HL):
            # qT/kT for head h: [D=128, T], q pre-scaled by 1/sqrt(D) via host
            qh = qk.tile([P, T], BF16, tag="qh")
            kh = qk.tile([P, T], BF16, tag="kh")
            for dst, w_d, b_sb in ((qh, wq_d, bq_sb), (kh, wk_d, bk_sb)):
                wc_t = wcol.tile([P, KT, P], BF16)
                nc.sync.dma_start(wc_t[:], w_d[h])
                for pair in range(2):
                    ps = mm.tile([P, 1024], F32, tag="mm")
                    for kt in range(KT):
                        for t2 in range(2):
                            tci = pair * 2 + t2
                            nc.tensor.matmul(
                                ps[:, t2 * 512:(t2 + 1) * 512],
                                wc_t[:, kt, :],
                                xt[:, kt, tci * 512:(tci + 1) * 512],
                                start=(kt == 0), stop=(kt == KT - 1),
                            )
                    nc.vector.tensor_tensor(
                        dst[:, pair * 1024:(pair + 1) * 1024], ps[:],
                        b_sb[:, h:h + 1].to_broadcast((P, 1024)),
                        AluOpType.add,
                    )

            # attention for head h, query chunks of 512; j-tiles in pairs so
            # exp/mask/evict run on [128, 1024] strips (2 PSUM banks)
            for c in range(TCH):
                jmax = 4 * c + 3
                npair = (jmax + 1) // 2
                ps_y = psy.tile([P, 512], F32, tag="y")
                ps_d = psd.tile([1, 512], F32)
                for jp in range(npair):
                    ps_s = mm.tile([P, 1024], F32, tag="mm")
                    for half in range(2):
                        jt = 2 * jp + half
                        nc.tensor.matmul(
                            ps_s[:, half * 512:(half + 1) * 512],
                            kh[:, jt * P:(jt + 1) * P],
                            qh[:, c * 512:(c + 1) * 512], start=True, stop=True,
                        )
                    pt = ppool.tile([P, 1024], BF16)
                    nc.scalar.activation(pt[:], ps_s[:], EXP)
                    if 2 * jp >= 4 * c:  # diagonal block: causal 0/1 masks
                        nc.vector.tensor_tensor(
                            pt[:], pt[:], mk_sb[:, jp - 2 * c, :], AluOpType.mult
                        )
                    for half in range(2):
                        jt = 2 * jp + half
                        nc.tensor.matmul(
                            ps_y[:], v_sb[:, jt, h * D:(h + 1) * D],
                            pt[:, half * 512:(half + 1) * 512],
                            start=(jt == 0), stop=(jt == jmax),
                        )
                        nc.tensor.matmul(
                            ps_d[:], ones_col[:],
                            pt[:, half * 512:(half + 1) * 512],
                            start=(jt == 0), stop=(jt == jmax),
                        )
                # Decouple normalization from the PE pipeline: evict both PSUM
                # accumulators to SBUF right away (frees psy/psd slots), then
                # broadcast the raw denominators with a K=1 matmul and do the
                # reciprocal full-width off the critical path.
                yu = spool.tile([P, 512], F32, tag="yu")
                nc.vector.tensor_copy(yu[:], ps_y[:])
                dn = spool.tile([1, 512], BF16, tag="dn")
                nc.scalar.copy(dn[:], ps_d[:])
                ps_b = psy.tile([P, 512], F32, tag="y", name="ps_b")
                nc.tensor.matmul(ps_b[:], ones_row[:], dn[:], start=True, stop=True)
                bc = spool.tile([P, 512], F32, tag="bc")
                nc.scalar.copy(bc[:], ps_b[:])
                rc = spool.tile([P, 512], F32, tag="rc")
                nc.vector.reciprocal(rc[:], bc[:])
                nc.vector.tensor_tensor(
                    yt[:, h, c * 512:(c + 1) * 512], yu[:], rc[:], AluOpType.mult
                )

        # stage D: partial out = y @ w_proj[hg rows]
        wp = big.tile([P, HL, C], BF16, tag="big")  # waits until xt is dead
        nc.sync.dma_start(wp[:], wp_d[:])
        for tt in range(KT):
            for pair in range(2):
                ps = mm.tile([P, 1024], F32, tag="mm")
                for hh in range(HL):
                    for c2 in range(2):
                        cc = pair * 2 + c2
                        nc.tensor.matmul(
                            ps[:, c2 * 512:(c2 + 1) * 512],
                            yt[:, hh, tt * P:(tt + 1) * P],
                            wp[:, hh, cc * 512:(cc + 1) * 512],
                            start=(hh == 0), stop=(hh == HL - 1),
                        )
                ot = opool.tile([P, 1024], F32)
                nc.vector.tensor_copy(ot[:], ps[:])
                nc.sync.dma_start(
                    out_d[tt * P:(tt + 1) * P, pair * 1024:(pair + 1) * 1024], ot[:]
                )

    nc.compile()
    return nc


def get_nc():
    if not _NC_CACHE:
        _NC_CACHE.append(build_nc())
    return _NC_CACHE[0]


def _bf(a):
    return np.ascontiguousarray(a).astype(ml_dtypes.bfloat16)


def _shard_inputs(x, w_attn, b_attn, w_proj):
    """Build the 8 per-core in_maps."""
    scale = 1.0 / math.sqrt(D)
    w_q, w_k, w_v = w_attn[:, :C], w_attn[:, C:2 * C], w_attn[:, 2 * C:]
    b_q, b_k, b_v = b_attn[:C], b_attn[C:2 * C], b_attn[2 * C:]

    # causal masks for the 4 j-tile positions within a 512 diagonal block,
    # paired as [2, 128, 1024]: pair 0 = (jt offset 0, 1), pair 1 = (2, 3)
    j = np.arange(P)[:, None]
    i = np.arange(512)[None, :]
    m4 = [(j + P * p <= i).astype(np.float32) for p in range(4)]
    masks = np.stack([np.concatenate([m4[0], m4[1]], axis=1),
                      np.concatenate([m4[2], m4[3]], axis=1)])
    masks_bf = _bf(masks.transpose(1, 0, 2))  # [128, 2, 1024]

    per_hg = {}
    for hg in range(HG):
        s = slice(hg * HL * D, (hg + 1) * HL * D)
        wq = _bf((w_q[:, s] * scale).reshape(KT, P, HL, D).transpose(2, 1, 0, 3))
        wk = _bf(w_k[:, s].reshape(KT, P, HL, D).transpose(2, 1, 0, 3))
        wv = _bf(w_v[:, s].reshape(KT, P, HL * D).transpose(1, 0, 2))
        wp = _bf(w_proj[s, :].reshape(HL, P, C).transpose(1, 0, 2))
        bq = np.ascontiguousarray(
            (b_q[s] * scale).reshape(HL, P).T).astype(np.float32)
        bk = np.ascontiguousarray(b_k[s].reshape(HL, P).T).astype(np.float32)
        bv = np.ascontiguousarray(
            np.broadcast_to(b_v[s], (P, HL * D))).astype(np.float32)
        per_hg[hg] = dict(wq=wq, wk=wk, wv=wv, wp=wp, bq=bq, bk=bk, bv=bv)

    in_maps = []
    for core in range(8):
        b, hg = core // HG, core % HG
        xt = _bf(x[b].T.reshape(KT, P, T).transpose(1, 0, 2))
        in_maps.append({"xt": xt, "masks": masks_bf, **per_hg[hg]})
    return in_maps


def kernel(x, w_attn, b_attn, w_proj, b_proj):
    global LAST_RESULTS
    x = np.asarray(x, dtype=np.float32)
    w_attn = np.asarray(w_attn, dtype=np.float32)
    b_attn = np.asarray(b_attn, dtype=np.float32)
    w_proj = np.asarray(w_proj, dtype=np.float32)
    b_proj = np.asarray(b_proj, dtype=np.float32)

    nc = get_nc()
    in_maps = _shard_inputs(x, w_attn, b_attn, w_proj)
    res = run_bass_kernel_spmd(nc, in_maps, core_ids=list(range(8)))
    LAST_RESULTS = res

    out = np.empty((B, T, C), dtype=np.float32)
    for b in range(B):
        out[b] = res.results[HG * b]["out"] + res.results[HG * b + 1]["out"]
        out[b] += b_proj[None, :]
    return out


# revision 18
# speedup vs baseline: 1.2093x; 1.0253x over previous
"""Causal self-attention (B=4, T=2048, C=2048, H=16, D=128) on 8 TRN2 NeuronCores.

Sharding: 8 shards = (batch b in 0..3) x (head-group hg in {0,1}, 8 heads each).
Each core computes qkv for its (batch, 8 heads), causal attention, and a
partial output projection y_hg @ w_proj[hg-rows]; the host sums the two
partials per batch and adds b_proj.

Device compute is bf16 (fp32 PSUM accumulation); inputs are cast/sliced/
pre-transposed on the host so the device never transposes anything:
  - x is shipped transposed (xT, [C, T] tiled as [128, 16, 2048]).
  - qT/kT computed as w.T @ xT (output [head-dim, T]) -> directly usable as
    matmul operands for S^T = k.T-tile @ qT.
  - softmax runs on S^T (keys on partitions): exp on ScalarE (no max
    subtraction needed; logits are O(1)), causal masking via precomputed 0/1
    mask tiles, denominators via ones-matmul column sums, normalization folded
    into the yT eviction with a K=1 broadcast matmul of the reciprocals.
  - v computed in natural [T, head-dim] layout (lhsT = xT tiles) so the AV
    matmul yT += v-block.T @ P^T needs no transpose either.
  - out = y @ w_proj with lhsT = yT tiles, rhs = w_proj slices.
1/sqrt(D) is folded into w_q (and b_q) on the host.
"""

import math
from contextlib import ExitStack

import ml_dtypes
import numpy as np

import concourse.bass as bass  # noqa: F401  (bass types used via tile/bacc)
import concourse.tile as tile
from concourse import bacc, mybir
from concourse.alu_op_type import AluOpType
from concourse.bass_utils import run_bass_kernel_spmd

P = 128
B, T, C, H, D = 4, 2048, 2048, 16, 128
HG = 2              # head groups (tensor-parallel factor); B * HG = 8 cores
HL = H // HG        # heads per core
KT = C // P         # 16 contraction tiles
TCH = T // 512      # 4 query chunks of 512
BF16 = mybir.dt.bfloat16
F32 = mybir.dt.float32
EXP = mybir.ActivationFunctionType.Exp

_NC_CACHE = []
LAST_RESULTS = None  # BassKernelResults of the most recent kernel() call


def build_nc():
    nc = bacc.Bacc("TRN2", target_bir_lowering=False, debug=False, num_devices=8)

    xt_d = nc.dram_tensor("xt", [P, KT, T], BF16, kind="ExternalInput")
    wq_d = nc.dram_tensor("wq", [HL, P, KT, P], BF16, kind="ExternalInput")
    wk_d = nc.dram_tensor("wk", [HL, P, KT, P], BF16, kind="ExternalInput")
    wv_d = nc.dram_tensor("wv", [P, KT, HL * D], BF16, kind="ExternalInput")
    wp_d = nc.dram_tensor("wp", [P, HL, C], BF16, kind="ExternalInput")
    bq_d = nc.dram_tensor("bq", [P, HL], F32, kind="ExternalInput")
    bk_d = nc.dram_tensor("bk", [P, HL], F32, kind="ExternalInput")
    bv_d = nc.dram_tensor("bv", [P, HL * D], F32, kind="ExternalInput")
    mk_d = nc.dram_tensor("masks", [P, 2, 1024], BF16, kind="ExternalInput")
    out_d = nc.dram_tensor("out", [T, C], F32, kind="ExternalOutput")

    with nc.allow_low_precision("bf16 attention kernel"), \
         tile.TileContext(nc) as tc, ExitStack() as ctx:
        consts = ctx.enter_context(tc.tile_pool(name="consts", bufs=1))
        big = ctx.enter_context(tc.tile_pool(name="big", bufs=1))      # xt -> wp
        vpool = ctx.enter_context(tc.tile_pool(name="v", bufs=1))
        mid = ctx.enter_context(tc.tile_pool(name="mid", bufs=1))      # wv -> yt
        qk = ctx.enter_context(tc.tile_pool(name="qk", bufs=2))
        wcol = ctx.enter_context(tc.tile_pool(name="wcol", bufs=3))
        ppool = ctx.enter_context(tc.tile_pool(name="p", bufs=6))
        spool = ctx.enter_context(tc.tile_pool(name="small", bufs=3))
        opool = ctx.enter_context(tc.tile_pool(name="o", bufs=3))
        # PSUM: two 2-bank strips (matmul pairs) + ps_y/ps_b pool + ps_d pool
        mm = ctx.enter_context(tc.tile_pool(name="mm", bufs=2, space="PSUM"))
        psy = ctx.enter_context(tc.tile_pool(name="psy", bufs=2, space="PSUM"))
        psd = ctx.enter_context(tc.tile_pool(name="psd", bufs=2, space="PSUM"))

        # constants
        bq_sb = consts.tile([P, HL], F32)
        nc.sync.dma_start(bq_sb[:], bq_d[:])
        bk_sb = consts.tile([P, HL], F32)
        nc.sync.dma_start(bk_sb[:], bk_d[:])
        bv_sb = consts.tile([P, HL * D], F32)
        nc.sync.dma_start(bv_sb[:], bv_d[:])
        mk_sb = consts.tile([P, 2, 1024], BF16)
        nc.sync.dma_start(mk_sb[:], mk_d[:])
        ones_col = consts.tile([P, 1], BF16)
        nc.vector.memset(ones_col[:], 1.0)
        ones_row = consts.tile([1, P], BF16)
        nc.vector.memset(ones_row[:], 1.0)

        # load wv first, then xT strips, so stage B can start as strips arrive
        wv = mid.tile([P, KT, HL * D], BF16, tag="mid")
        nc.sync.dma_start(wv[:], wv_d[:])
        xt = big.tile([P, KT, T], BF16, tag="big")
        for kt in range(KT):
            nc.sync.dma_start(xt[:, kt, :], xt_d[:, kt, :])

        def emit_qk(h):
            """qT/kT for head h: [D=128, T]; q pre-scaled by 1/sqrt(D) on host."""
            qh = qk.tile([P, T], BF16, tag="qh", name=f"qh{h}")
            kh = qk.tile([P, T], BF16, tag="kh", name=f"kh{h}")
            for dst, w_d, b_sb in ((qh, wq_d, bq_sb), (kh, wk_d, bk_sb)):
                wc_t = wcol.tile([P, KT, P], BF16, tag="wcol", name=f"wcol{h}")
                nc.sync.dma_start(wc_t[:], w_d[h])
                for pair in range(2):
                    ps = mm.tile([P, 1024], F32, tag="mm", name=f"qkps{h}")
                    for kt in range(KT):
                        for t2 in range(2):
                            tci = pair * 2 + t2
                            nc.tensor.matmul(
                                ps[:, t2 * 512:(t2 + 1) * 512],
                                wc_t[:, kt, :],
                                xt[:, kt, tci * 512:(tci + 1) * 512],
                                start=(kt == 0), stop=(kt == KT - 1),
                            )
                    nc.vector.tensor_tensor(
                        dst[:, pair * 1024:(pair + 1) * 1024], ps[:],
                        b_sb[:, h:h + 1].to_broadcast((P, 1024)),
                        AluOpType.add,
                    )
            return qh, kh

        # head 0's q/k first: its matmuls consume xT strips as they arrive,
        # filling the initial DMA window before stage B needs the full xT
        qk_head0 = emit_qk(0)

        # stage B: v = x @ w_v, natural layout v_sb[p=T within tt, tt, hl*D]
        v_sb = vpool.tile([P, KT, HL * D], BF16)
        for tt in range(KT):
            ps = mm.tile([P, 1024], F32, tag="mm")
            for kt in range(KT):
                for wc in range(2):
                    nc.tensor.matmul(
                        ps[:, wc * 512:(wc + 1) * 512],
                        xt[:, kt, tt * P:(tt + 1) * P],
                        wv[:, kt, wc * 512:(wc + 1) * 512],
                        start=(kt == 0), stop=(kt == KT - 1),
                    )
            nc.vector.tensor_tensor(
                v_sb[:, tt, :], ps[:], bv_sb[:], AluOpType.add,
            )

        yt = mid.tile([P, HL, T], BF16, tag="mid")  # waits until wv is dead

        for h in range(HL):
            qh, kh = qk_head0 if h == 0 else emit_qk(h)

            # attention for head h, query chunks of 512; j-tiles in pairs so
            # exp/mask/evict run on [128, 1024] strips (2 PSUM banks)
            for c in range(TCH):
                jmax = 4 * c + 3
                npair = (jmax + 1) // 2
                ps_y = psy.tile([P, 512], F32, tag="y")
                ps_d = psd.tile([1, 512], F32)
                for jp in range(npair):
                    ps_s = mm.tile([P, 1024], F32, tag="mm")
                    for half in range(2):
                        jt = 2 * jp + half
                        nc.tensor.matmul(
                            ps_s[:, half * 512:(half + 1) * 512],
                            kh[:, jt * P:(jt + 1) * P],
                            qh[:, c * 512:(c + 1) * 512], start=True, stop=True,
                        )
                    pt = ppool.tile([P, 1024], BF16)
                    nc.scalar.activation(pt[:], ps_s[:], EXP)
                    if 2 * jp >= 4 * c:  # diagonal block: causal 0/1 masks
                        nc.vector.tensor_tensor(
                            pt[:], pt[:], mk_sb[:, jp - 2 * c, :], AluOpType.mult
                        )
                    for half in range(2):
                        jt = 2 * jp + half
                        nc.tensor.matmul(
                            ps_y[:], v_sb[:, jt, h * D:(h + 1) * D],
                            pt[:, half * 512:(half + 1) * 512],
                            start=(jt == 0), stop=(jt == jmax),
                        )
                        nc.tensor.matmul(
                            ps_d[:], ones_col[:],
                            pt[:, half * 512:(half + 1) * 512],
                            start=(jt == 0), stop=(jt == jmax),
                        )
                # Decouple normalization from the PE pipeline: evict both PSUM
                # accumulators right away (frees psy/psd slots), take fast
                # reciprocals of the [1,512] denominators, broadcast them with
                # a K=1 matmul, and scale the unnormalized yT chunk.
                yu = spool.tile([P, 512], F32, tag="yu")
                nc.vector.tensor_copy(yu[:], ps_y[:])
                rf = spool.tile([1, 512], F32, tag="rf")
                nc.vector.reciprocal_approx_fast(rf[:], ps_d[:])
                rb = spool.tile([1, 512], BF16, tag="rb")
                nc.scalar.copy(rb[:], rf[:])
                ps_b = psy.tile([P, 512], F32, tag="y", name="ps_b")
                nc.tensor.matmul(ps_b[:], ones_row[:], rb[:], start=True, stop=True)
                nc.vector.tensor_tensor(
                    yt[:, h, c * 512:(c + 1) * 512], yu[:], ps_b[:], AluOpType.mult
                )

        # stage D: partial out = y @ w_proj[hg rows]
        wp = big.tile([P, HL, C], BF16, tag="big")  # waits until xt is dead
        nc.sync.dma_start(wp[:], wp_d[:])
        for tt in range(KT):
            for pair in range(2):
                ps = mm.tile([P, 1024], F32, tag="mm")
                for hh in range(HL):
                    for c2 in range(2):
                        cc = pair * 2 + c2
                        nc.tensor.matmul(
                            ps[:, c2 * 512:(c2 + 1) * 512],
                            yt[:, hh, tt * P:(tt + 1) * P],
                            wp[:, hh, cc * 512:(cc + 1) * 512],
                            start=(hh == 0), stop=(hh == HL - 1),
                        )
                ot = opool.tile([P, 1024], F32)
                nc.vector.tensor_copy(ot[:], ps[:])
                nc.sync.dma_start(
                    out_d[tt * P:(tt + 1) * P, pair * 1024:(pair + 1) * 1024], ot[:]
                )

    nc.compile()
    return nc


def get_nc():
    if not _NC_CACHE:
        _NC_CACHE.append(build_nc())
    return _NC_CACHE[0]


def _bf(a):
    return np.ascontiguousarray(a).astype(ml_dtypes.bfloat16)


def _shard_inputs(x, w_attn, b_attn, w_proj):
    """Build the 8 per-core in_maps."""
    scale = 1.0 / math.sqrt(D)
    w_q, w_k, w_v = w_attn[:, :C], w_attn[:, C:2 * C], w_attn[:, 2 * C:]
    b_q, b_k, b_v = b_attn[:C], b_attn[C:2 * C], b_attn[2 * C:]

    # causal masks for the 4 j-tile positions within a 512 diagonal block,
    # paired as [2, 128, 1024]: pair 0 = (jt offset 0, 1), pair 1 = (2, 3)
    j = np.arange(P)[:, None]
    i = np.arange(512)[None, :]
    m4 = [(j + P * p <= i).astype(np.float32) for p in range(4)]
    masks = np.stack([np.concatenate([m4[0], m4[1]], axis=1),
                      np.concatenate([m4[2], m4[3]], axis=1)])
    masks_bf = _bf(masks.transpose(1, 0, 2))  # [128, 2, 1024]

    per_hg = {}
    for hg in range(HG):
        s = slice(hg * HL * D, (hg + 1) * HL * D)
        wq = _bf((w_q[:, s] * scale).reshape(KT, P, HL, D).transpose(2, 1, 0, 3))
        wk = _bf(w_k[:, s].reshape(KT, P, HL, D).transpose(2, 1, 0, 3))
        wv = _bf(w_v[:, s].reshape(KT, P, HL * D).transpose(1, 0, 2))
        wp = _bf(w_proj[s, :].reshape(HL, P, C).transpose(1, 0, 2))
        bq = np.ascontiguousarray(
            (b_q[s] * scale).reshape(HL, P).T).astype(np.float32)
        bk = np.ascontiguousarray(b_k[s].reshape(HL, P).T).astype(np.float32)
        bv = np.ascontiguousarray(
            np.broadcast_to(b_v[s], (P, HL * D))).astype(np.float32)
        per_hg[hg] = dict(wq=wq, wk=wk, wv=wv, wp=wp, bq=bq, bk=bk, bv=bv)

    in_maps = []
    for core in range(8):
        b, hg = core // HG, core % HG
        xt = _bf(x[b].T.reshape(KT, P, T).transpose(1, 0, 2))
        in_maps.append({"xt": xt, "masks": masks_bf, **per_hg[hg]})
    return in_maps


def kernel(x, w_attn, b_attn, w_proj, b_proj):
    global LAST_RESULTS
    x = np.asarray(x, dtype=np.float32)
    w_attn = np.asarray(w_attn, dtype=np.float32)
    b_attn = np.asarray(b_attn, dtype=np.float32)
    w_proj = np.asarray(w_proj, dtype=np.float32)
    b_proj = np.asarray(b_proj, dtype=np.float32)

    nc = get_nc()
    in_maps = _shard_inputs(x, w_attn, b_attn, w_proj)
    res = run_bass_kernel_spmd(nc, in_maps, core_ids=list(range(8)))
    LAST_RESULTS = res

    out = np.empty((B, T, C), dtype=np.float32)
    for b in range(B):
        out[b] = res.results[HG * b]["out"] + res.results[HG * b + 1]["out"]
        out[b] += b_proj[None, :]
    return out


# revision 19
# speedup vs baseline: 1.2986x; 1.0739x over previous
"""Causal self-attention (B=4, T=2048, C=2048, H=16, D=128) on 8 TRN2 NeuronCores.

Sharding: 8 shards = (batch b in 0..3) x (head-group hg in {0,1}, 8 heads each).
Each core computes qkv for its (batch, 8 heads), causal attention, and a
partial output projection y_hg @ w_proj[hg-rows]; the host sums the two
partials per batch and adds b_proj.

Device compute is bf16 (fp32 PSUM accumulation); inputs are cast/sliced/
pre-transposed on the host so the device never transposes anything:
  - x is shipped transposed (xT, [C, T] tiled as [128, 16, 2048]).
  - qT/kT computed as w.T @ xT (output [head-dim, T]) -> directly usable as
    matmul operands for S^T = k.T-tile @ qT.
  - softmax runs on S^T (keys on partitions): exp on ScalarE (no max
    subtraction needed; logits are O(1)), causal masking via precomputed 0/1
    mask tiles, denominators via ones-matmul column sums, normalization folded
    into the yT eviction with a K=1 broadcast matmul of the reciprocals.
  - v computed in natural [T, head-dim] layout (lhsT = xT tiles) so the AV
    matmul yT += v-block.T @ P^T needs no transpose either.
  - out = y @ w_proj with lhsT = yT tiles, rhs = w_proj slices.
1/sqrt(D) is folded into w_q (and b_q) on the host.
"""

import math
from contextlib import ExitStack

import ml_dtypes
import numpy as np

import concourse.bass as bass  # noqa: F401  (bass types used via tile/bacc)
import concourse.tile as tile
from concourse import bacc, mybir
from concourse.alu_op_type import AluOpType
from concourse.bass_utils import run_bass_kernel_spmd

P = 128
B, T, C, H, D = 4, 2048, 2048, 16, 128
HG = 2              # head groups (tensor-parallel factor); B * HG = 8 cores
HL = H // HG        # heads per core
KT = C // P         # 16 contraction tiles
TCH = T // 512      # 4 query chunks of 512
BF16 = mybir.dt.bfloat16
F32 = mybir.dt.float32
EXP = mybir.ActivationFunctionType.Exp

_NC_CACHE = []
LAST_RESULTS = None  # BassKernelResults of the most recent kernel() call


def build_nc():
    nc = bacc.Bacc("TRN2", target_bir_lowering=False, debug=False, num_devices=8)

    xt_d = nc.dram_tensor("xt", [P, KT, T], BF16, kind="ExternalInput")
    wq_d = nc.dram_tensor("wq", [HL, P, KT, P], BF16, kind="ExternalInput")
    wk_d = nc.dram_tensor("wk", [HL, P, KT, P], BF16, kind="ExternalInput")
    wv_d = nc.dram_tensor("wv", [P, KT, HL * D], BF16, kind="ExternalInput")
    wp_d = nc.dram_tensor("wp", [P, HL, C], BF16, kind="ExternalInput")
    bq_d = nc.dram_tensor("bq", [P, HL], F32, kind="ExternalInput")
    bk_d = nc.dram_tensor("bk", [P, HL], F32, kind="ExternalInput")
    bv_d = nc.dram_tensor("bv", [P, HL * D], F32, kind="ExternalInput")
    mk_d = nc.dram_tensor("masks", [P, 2, 1024], BF16, kind="ExternalInput")
    out_d = nc.dram_tensor("out", [T, C], F32, kind="ExternalOutput")

    with nc.allow_low_precision("bf16 attention kernel"), \
         tile.TileContext(nc) as tc, ExitStack() as ctx:
        consts = ctx.enter_context(tc.tile_pool(name="consts", bufs=1))
        big = ctx.enter_context(tc.tile_pool(name="big", bufs=1))      # xt -> wp
        vpool = ctx.enter_context(tc.tile_pool(name="v", bufs=1))
        mid = ctx.enter_context(tc.tile_pool(name="mid", bufs=1))      # wv -> yt
        qk = ctx.enter_context(tc.tile_pool(name="qk", bufs=2))
        wcol = ctx.enter_context(tc.tile_pool(name="wcol", bufs=3))
        ppool = ctx.enter_context(tc.tile_pool(name="p", bufs=6))
        spool = ctx.enter_context(tc.tile_pool(name="small", bufs=3))
        opool = ctx.enter_context(tc.tile_pool(name="o", bufs=3))
        # PSUM: two 2-bank strips (matmul pairs) + ps_y/ps_b pool + ps_d pool
        mm = ctx.enter_context(tc.tile_pool(name="mm", bufs=3, space="PSUM"))
        psy = ctx.enter_context(tc.tile_pool(name="psy", bufs=1, space="PSUM"))
        psd = ctx.enter_context(tc.tile_pool(name="psd", bufs=1, space="PSUM"))

        # constants
        bq_sb = consts.tile([P, HL], F32)
        nc.sync.dma_start(bq_sb[:], bq_d[:])
        bk_sb = consts.tile([P, HL], F32)
        nc.sync.dma_start(bk_sb[:], bk_d[:])
        bv_sb = consts.tile([P, HL * D], F32)
        nc.gpsimd.dma_start(bv_sb[:], bv_d[:])
        mk_sb = consts.tile([P, 2, 1024], BF16)
        nc.gpsimd.dma_start(mk_sb[:], mk_d[:])
        ones_col = consts.tile([P, 1], BF16)
        nc.vector.memset(ones_col[:], 1.0)
        ones_row = consts.tile([1, P], BF16)
        nc.vector.memset(ones_row[:], 1.0)

        # xT strips first (head-0 q/k consumes them as they arrive), wv after
        # (stage B runs once head-0 q/k is done)
        xt = big.tile([P, KT, T], BF16, tag="big")
        for kt in range(KT):
            nc.sync.dma_start(xt[:, kt, :], xt_d[:, kt, :])
        wv = mid.tile([P, KT, HL * D], BF16, tag="mid")
        nc.sync.dma_start(wv[:], wv_d[:])

        def emit_qk(h):
            """qT/kT for head h: [D=128, T]; q pre-scaled by 1/sqrt(D) on host."""
            qh = qk.tile([P, T], BF16, tag="qh", name=f"qh{h}")
            kh = qk.tile([P, T], BF16, tag="kh", name=f"kh{h}")
            for dst, w_d, b_sb in ((qh, wq_d, bq_sb), (kh, wk_d, bk_sb)):
                wc_t = wcol.tile([P, KT, P], BF16, tag="wcol", name=f"wcol{h}")
                nc.gpsimd.dma_start(wc_t[:], w_d[h])
                for pair in range(2):
                    ps = mm.tile([P, 1024], F32, tag="mm", name=f"qkps{h}")
                    for kt in range(KT):
                        for t2 in range(2):
                            tci = pair * 2 + t2
                            nc.tensor.matmul(
                                ps[:, t2 * 512:(t2 + 1) * 512],
                                wc_t[:, kt, :],
                                xt[:, kt, tci * 512:(tci + 1) * 512],
                                start=(kt == 0), stop=(kt == KT - 1),
                            )
                    nc.vector.tensor_tensor(
                        dst[:, pair * 1024:(pair + 1) * 1024], ps[:],
                        b_sb[:, h:h + 1].to_broadcast((P, 1024)),
                        AluOpType.add,
                    )
            return qh, kh

        # head 0's q/k first: its matmuls consume xT strips as they arrive,
        # filling the initial DMA window before stage B needs the full xT
        qk_head0 = emit_qk(0)

        # stage B: v = x @ w_v, natural layout v_sb[p=T within tt, tt, hl*D]
        v_sb = vpool.tile([P, KT, HL * D], BF16)
        for tt in range(KT):
            ps = mm.tile([P, 1024], F32, tag="mm")
            for kt in range(KT):
                for wc in range(2):
                    nc.tensor.matmul(
                        ps[:, wc * 512:(wc + 1) * 512],
                        xt[:, kt, tt * P:(tt + 1) * P],
                        wv[:, kt, wc * 512:(wc + 1) * 512],
                        start=(kt == 0), stop=(kt == KT - 1),
                    )
            nc.vector.tensor_tensor(
                v_sb[:, tt, :], ps[:], bv_sb[:], AluOpType.add,
            )

        yt = mid.tile([P, HL, T], BF16, tag="mid")  # waits until wv is dead

        for h in range(HL):
            qh, kh = qk_head0 if h == 0 else emit_qk(h)

            # attention for head h, query chunks of 512; j-tiles in pairs so
            # exp/mask/evict run on [128, 1024] strips (2 PSUM banks)
            for c in range(TCH):
                jmax = 4 * c + 3
                npair = (jmax + 1) // 2
                ps_y = psy.tile([P, 512], F32, tag="y")
                ps_d = psd.tile([1, 512], F32)
                for jp in range(npair):
                    ps_s = mm.tile([P, 1024], F32, tag="mm")
                    for half in range(2):
                        jt = 2 * jp + half
                        nc.tensor.matmul(
                            ps_s[:, half * 512:(half + 1) * 512],
                            kh[:, jt * P:(jt + 1) * P],
                            qh[:, c * 512:(c + 1) * 512], start=True, stop=True,
                        )
                    pt = ppool.tile([P, 1024], BF16)
                    nc.scalar.activation(pt[:], ps_s[:], EXP)
                    if 2 * jp >= 4 * c:  # diagonal block: causal 0/1 masks
                        nc.vector.tensor_tensor(
                            pt[:], pt[:], mk_sb[:, jp - 2 * c, :], AluOpType.mult
                        )
                    for half in range(2):
                        jt = 2 * jp + half
                        nc.tensor.matmul(
                            ps_y[:], v_sb[:, jt, h * D:(h + 1) * D],
                            pt[:, half * 512:(half + 1) * 512],
                            start=(jt == 0), stop=(jt == jmax),
                        )
                        nc.tensor.matmul(
                            ps_d[:], ones_col[:],
                            pt[:, half * 512:(half + 1) * 512],
                            start=(jt == 0), stop=(jt == jmax),
                        )
                # Decouple normalization from the PE pipeline: evict both PSUM
                # accumulators right away (frees psy/psd slots), take fast
                # reciprocals of the [1,512] denominators, broadcast them with
                # a K=1 matmul, and scale the unnormalized yT chunk.
                yu = spool.tile([P, 512], F32, tag="yu")
                nc.vector.tensor_copy(yu[:], ps_y[:])
                rf = spool.tile([1, 512], F32, tag="rf")
                nc.vector.reciprocal_approx_fast(rf[:], ps_d[:])
                rb = spool.tile([1, 512], BF16, tag="rb")
                nc.scalar.copy(rb[:], rf[:])
                ps_b = psy.tile([P, 512], F32, tag="y", name="ps_b")
                nc.tensor.matmul(ps_b[:], ones_row[:], rb[:], start=True, stop=True)
                nc.vector.tensor_tensor(
                    yt[:, h, c * 512:(c + 1) * 512], yu[:], ps_b[:], AluOpType.mult
                )

        # stage D: partial out = y @ w_proj[hg rows]
        wp = big.tile([P, HL, C], BF16, tag="big")  # waits until xt is dead
        nc.sync.dma_start(wp[:], wp_d[:])
        for tt in range(KT):
            for pair in range(2):
                ps = mm.tile([P, 1024], F32, tag="mm")
                for hh in range(HL):
                    for c2 in range(2):
                        cc = pair * 2 + c2
                        nc.tensor.matmul(
                            ps[:, c2 * 512:(c2 + 1) * 512],
                            yt[:, hh, tt * P:(tt + 1) * P],
                            wp[:, hh, cc * 512:(cc + 1) * 512],
                            start=(hh == 0), stop=(hh == HL - 1),
                        )
                ot = opool.tile([P, 1024], F32)
                nc.vector.tensor_copy(ot[:], ps[:])
                nc.sync.dma_start(
                    out_d[tt * P:(tt + 1) * P, pair * 1024:(pair + 1) * 1024], ot[:]
                )

    nc.compile()
    return nc


def get_nc():
    if not _NC_CACHE:
        _NC_CACHE.append(build_nc())
    return _NC_CACHE[0]


def _bf(a):
    return np.ascontiguousarray(a).astype(ml_dtypes.bfloat16)


def _shard_inputs(x, w_attn, b_attn, w_proj):
    """Build the 8 per-core in_maps."""
    scale = 1.0 / math.sqrt(D)
    w_q, w_k, w_v = w_attn[:, :C], w_attn[:, C:2 * C], w_attn[:, 2 * C:]
    b_q, b_k, b_v = b_attn[:C], b_attn[C:2 * C], b_attn[2 * C:]

    # causal masks for the 4 j-tile positions within a 512 diagonal block,
    # paired as [2, 128, 1024]: pair 0 = (jt offset 0, 1), pair 1 = (2, 3)
    j = np.arange(P)[:, None]
    i = np.arange(512)[None, :]
    m4 = [(j + P * p <= i).astype(np.float32) for p in range(4)]
    masks = np.stack([np.concatenate([m4[0], m4[1]], axis=1),
                      np.concatenate([m4[2], m4[3]], axis=1)])
    masks_bf = _bf(masks.transpose(1, 0, 2))  # [128, 2, 1024]

    per_hg = {}
    for hg in range(HG):
        s = slice(hg * HL * D, (hg + 1) * HL * D)
        wq = _bf((w_q[:, s] * scale).reshape(KT, P, HL, D).transpose(2, 1, 0, 3))
        wk = _bf(w_k[:, s].reshape(KT, P, HL, D).transpose(2, 1, 0, 3))
        wv = _bf(w_v[:, s].reshape(KT, P, HL * D).transpose(1, 0, 2))
        wp = _bf(w_proj[s, :].reshape(HL, P, C).transpose(1, 0, 2))
        bq = np.ascontiguousarray(
            (b_q[s] * scale).reshape(HL, P).T).astype(np.float32)
        bk = np.ascontiguousarray(b_k[s].reshape(HL, P).T).astype(np.float32)
        bv = np.ascontiguousarray(
            np.broadcast_to(b_v[s], (P, HL * D))).astype(np.float32)
        per_hg[hg] = dict(wq=wq, wk=wk, wv=wv, wp=wp, bq=bq, bk=bk, bv=bv)

    in_maps = []
    for core in range(8):
        b, hg = core // HG, core % HG
        xt = _bf(x[b].T.reshape(KT, P, T).transpose(1, 0, 2))
        in_maps.append({"xt": xt, "masks": masks_bf, **per_hg[hg]})
    return in_maps


def kernel(x, w_attn, b_attn, w_proj, b_proj):
    global LAST_RESULTS
    x = np.asarray(x, dtype=np.float32)
    w_attn = np.asarray(w_attn, dtype=np.float32)
    b_attn = np.asarray(b_attn, dtype=np.float32)
    w_proj = np.asarray(w_proj, dtype=np.float32)
    b_proj = np.asarray(b_proj, dtype=np.float32)

    nc = get_nc()
    in_maps = _shard_inputs(x, w_attn, b_attn, w_proj)
    res = run_bass_kernel_spmd(nc, in_maps, core_ids=list(range(8)))
    LAST_RESULTS = res

    out = np.empty((B, T, C), dtype=np.float32)
    for b in range(B):
        out[b] = res.results[HG * b]["out"] + res.results[HG * b + 1]["out"]
        out[b] += b_proj[None, :]
    return out
